# revision 14
# baseline (speedup 1.0000x reference)
"""Trainium2 Bass kernel for nn_MixedSparseGatedMLP (QLoRA-style NF4 gated MLP).

  y1 = x @ dequant(g).T + (x @ g_lora_a) @ g_lora_b
  y2 = x @ dequant(u).T + (x @ u_lora_a) @ u_lora_b
  y3 = (relu(y1)*y2) @ dequant(d).T + ((relu(y1)*y2) @ d_lora_a) @ d_lora_b

Strategy (Megatron tensor-parallel over d_ff across 8 cores):
 - d_ff padded 11008 -> 11264 = 8 * 1408; each core owns a 1408-wide shard of
   gate/up output channels (and the matching rows of the down projection).
   Padded channels carry absmax = 0 so they contribute exactly zero.
 - NF4 dequant on device: the 16-entry codebook is approximated by
   s1*ln(c+a) - s2*ln(b-c) + d0 + A1*sin(B1 c+P1) + A2*tanh(B2 c+P2)
   (max abs err 1.4e-3, rel rms 0.12%) evaluated on the ScalarEngine with
   4 activation passes + 5 VectorEngine combine passes, entirely overlapped
   with TensorEngine GEMMs. absmax scaling via partition-broadcast DMA.
 - Codes are uploaded pre-transposed-ready as int16; transposing DMA loads
   produce codes^T tiles so dequant directly yields W' in [in, out] layout
   (the layout the PE needs) - no on-device weight transpose.
 - LoRA adapters fold in as extra K=16 accumulation matmuls (concat trick).
 - fp16 on-device compute (PSUM accumulates fp32); each core returns its
   partial y3 in fp16; host reduces the 8 partials in fp32.
"""

import sys

sys.path.insert(0, "/opt/trn_rl_repo")

from contextlib import ExitStack

import numpy as np

import concourse.bass as bass
import concourse.mybir as mybir
import concourse.tile as tile
from concourse import bacc

F16 = mybir.dt.float16
F32 = mybir.dt.float32
I16 = mybir.dt.int16
AF = mybir.ActivationFunctionType
OP = mybir.AluOpType

# fitted NF4 approximation params (logit + sin + tanh), see module docstring
S1, LA, S2, LB, D0, A1, B1, P1, A2, B2, P2 = [
    3.20329446e-01, 6.94421837e-01, 3.37825232e-01, 1.58366395e+01,
    6.40217907e-02, 3.23956362e-02, 3.36126831e-01, -2.12588407e+00,
    -1.35181599e-02, 7.56234953e-01, -6.20180727e+00]

BLOCK = 64
NCORES = 8


def build_nc(N_TOK, DM, JS, NB):
    """One-core SPMD program. JS = per-core d_ff shard width, NB = token block.

    Device inputs (per core):
      x16   [N_TOK, DM] f16      - tokens (replicated)
      gc,uc [JS, DM]   i16       - gate/up NF4 codes, shard rows
      gam,uam [DM//BLOCK, JS] f16 - absmax transposed
      dc    [DM, JS]   i16       - down codes, shard cols
      dam   [JS//BLOCK, DM] f16  - down absmax transposed
      gla,ula [DM, 16] f16; glb,ulb [16, JS] f16
      dla   [JS, 16]   f16; dlb [16, DM] f16
    Output: y3p [N_TOK, DM] f16 - this core's partial down projection
    """
    IT = DM // 128          # i tiles (contraction dim of gate/up)
    JT = JS // 128          # j tiles (shard channels)
    NBLK = N_TOK // NB      # token blocks
    NF = min(512, NB)       # matmul moving free dim
    NH = NB // NF           # halves per block
    NS = NB // 128          # 128-token subtiles per block
    MT = DM // NF           # output column tiles (down)
    DQF = min(1024, DM)     # free-dim chunk for down dequant
    DQC = DM // DQF
    BPT = 128 // BLOCK      # absmax rows per 128-row tile

    nc = bacc.Bacc("TRN2", target_bir_lowering=False, debug=False,
                   num_devices=NCORES)

    x16 = nc.dram_tensor("x16", [N_TOK, DM], F16, kind="ExternalInput")
    gc = nc.dram_tensor("gc", [JS, DM], I16, kind="ExternalInput")
    gam = nc.dram_tensor("gam", [DM // BLOCK, JS], F16, kind="ExternalInput")
    uc = nc.dram_tensor("uc", [JS, DM], I16, kind="ExternalInput")
    uam = nc.dram_tensor("uam", [DM // BLOCK, JS], F16, kind="ExternalInput")
    dc = nc.dram_tensor("dc", [DM, JS], I16, kind="ExternalInput")
    dam = nc.dram_tensor("dam", [JS // BLOCK, DM], F16, kind="ExternalInput")
    gla = nc.dram_tensor("gla", [DM, 16], F16, kind="ExternalInput")
    glb = nc.dram_tensor("glb", [16, JS], F16, kind="ExternalInput")
    ula = nc.dram_tensor("ula", [DM, 16], F16, kind="ExternalInput")
    ulb = nc.dram_tensor("ulb", [16, JS], F16, kind="ExternalInput")
    dla = nc.dram_tensor("dla", [JS, 16], F16, kind="ExternalInput")
    dlb = nc.dram_tensor("dlb", [16, DM], F16, kind="ExternalInput")
    y3p = nc.dram_tensor("y3p", [N_TOK, DM], F16, kind="ExternalOutput")

    with tile.TileContext(nc) as tc, ExitStack() as ctx:
        dram = ctx.enter_context(tc.tile_pool(name="dram", bufs=1, space="DRAM"))
        small = ctx.enter_context(tc.tile_pool(name="small", bufs=1))

        wgd = dram.tile([DM, JS], F16)   # W' gate  [in, out]
        wud = dram.tile([DM, JS], F16)   # W' up    [in, out]
        wdd = dram.tile([JS, DM], F16)   # W' down  [in(j), out(m)]

        # activation bias constants
        biases = small.tile([128, 4], F32)
        for k, v in enumerate([LA, LB, P1, P2]):
            nc.vector.memset(biases[:, k:k + 1], float(v))
        b_la, b_lb, b_p1, b_p2 = (biases[:, k:k + 1] for k in range(4))

        # resident LoRA factors
        gla_s = small.tile([128, IT, 16], F16)
        nc.sync.dma_start(gla_s[:], gla.rearrange("(t p) r -> p t r", p=128))
        ula_s = small.tile([128, IT, 16], F16)
        nc.sync.dma_start(ula_s[:], ula.rearrange("(t p) r -> p t r", p=128))
        dla_s = small.tile([128, JT, 16], F16)
        nc.sync.dma_start(dla_s[:], dla.rearrange("(t p) r -> p t r", p=128))
        glb_s = small.tile([16, JS], F16)
        nc.sync.dma_start(glb_s[:], glb[:])
        ulb_s = small.tile([16, JS], F16)
        nc.sync.dma_start(ulb_s[:], ulb[:])
        dlb_s = small.tile([16, DM], F16)
        nc.sync.dma_start(dlb_s[:], dlb[:])

        # ---- dequant: codes^T tile [128, F] -> W' tile, written to DRAM ----
        def dequant_tile(pool, codes_dram, am_dram, w_dram, t, F, f0):
            """rows [128t, 128t+128) of W' (= input-dim), free cols [f0, f0+F)"""
            ct = pool.tile([128, F], I16, tag="ct")
            nc.sync.dma_start_transpose(
                ct[:], codes_dram[f0:f0 + F, 128 * t:128 * (t + 1)])
            am_b = pool.tile([128, F], F16, tag="am_b")
            for b in range(BPT):
                nc.gpsimd.dma_start(
                    am_b[BLOCK * b:BLOCK * (b + 1), :],
                    am_dram[BPT * t + b:BPT * t + b + 1, f0:f0 + F]
                    .broadcast_to([BLOCK, F]))
            u1 = pool.tile([128, F], F16, tag="u1")
            u2 = pool.tile([128, F], F16, tag="u2")
            u3 = pool.tile([128, F], F16, tag="u3")
            u4 = pool.tile([128, F], F16, tag="u4")
            nc.scalar.activation(u1[:], ct[:], AF.Ln, bias=b_la, scale=1.0)
            nc.scalar.activation(u2[:], ct[:], AF.Ln, bias=b_lb, scale=-1.0)
            nc.scalar.activation(u3[:], ct[:], AF.Sin, bias=b_p1, scale=float(B1))
            nc.scalar.activation(u4[:], ct[:], AF.Tanh, bias=b_p2, scale=float(B2))
            t1 = pool.tile([128, F], F16, tag="t1")
            nc.vector.scalar_tensor_tensor(
                t1[:], u1[:], float(S1 / S2), u2[:], OP.mult, OP.subtract)
            t2 = pool.tile([128, F], F16, tag="t2")
            nc.vector.scalar_tensor_tensor(
                t2[:], u3[:], float(A1 / A2), u4[:], OP.mult, OP.add)
            t3 = pool.tile([128, F], F16, tag="t3")
            nc.vector.scalar_tensor_tensor(
                t3[:], t1[:], float(S2 / A2), t2[:], OP.mult, OP.add)
            t4 = pool.tile([128, F], F16, tag="t4")
            nc.vector.tensor_scalar(
                t4[:], t3[:], float(A2), float(D0), OP.mult, OP.add)
            wt = pool.tile([128, F], F16, tag="wt")
            nc.vector.tensor_tensor(wt[:], t4[:], am_b[:], OP.mult)
            nc.gpsimd.dma_start(w_dram[128 * t:128 * (t + 1), f0:f0 + F], wt[:])

        with tc.tile_pool(name="deq", bufs=3) as deq:
            # gate/up emitted in the order the GEMM consumes them: j-chunk
            # outer (group of <=512 cols), i-tile inner, gate then up
            f0 = 0
            while f0 < JS:
                F = min(512, JS - f0)
                for t in range(IT):
                    dequant_tile(deq, gc, gam, wgd, t, F, f0)
                for t in range(IT):
                    dequant_tile(deq, uc, uam, wud, t, F, f0)
                f0 += F
            # down: m-chunk outer (matches per-block m loop), j-tile inner
            f0 = 0
            while f0 < DM:
                F = min(512, DM - f0)
                for t in range(JT):
                    dequant_tile(deq, dc, dam, wdd, t, F, f0)
                f0 += F

        # ---- GEMM chain ----
        xblk = ctx.enter_context(tc.tile_pool(name="xblk", bufs=1))
        x3blk = ctx.enter_context(tc.tile_pool(name="x3blk", bufs=1))
        rblk = ctx.enter_context(tc.tile_pool(name="rblk", bufs=1))
        wstr = ctx.enter_context(tc.tile_pool(name="wstr", bufs=3))
        wdstr = ctx.enter_context(tc.tile_pool(name="wdstr", bufs=JT + 2))
        evac = ctx.enter_context(tc.tile_pool(name="evac", bufs=4))
        psum = ctx.enter_context(tc.tile_pool(name="psum", bufs=1, space="PSUM"))

        # all PSUM usage shares one 8-slot tag family (one bank each)
        psum_idx = [0]

        def ptile(idx, p=128):
            return psum.tile([128, NF], F32, tag=f"pp{idx}", name=f"pp{idx}")[:p]

        # j-tile groups of 2 (PSUM: 2 j x 2 halves x {gate,up} = 8 banks)
        JG = []
        j0 = 0
        while j0 < JT:
            jn = min(2, JT - j0)
            JG.append((j0, jn))
            j0 += jn

        for nb in range(NBLK):
            n0 = nb * NB
            # resident x^T tiles for this block
            xts = []
            for i in range(IT):
                xt = xblk.tile([128, NB], F16, tag=f"xt{i}", name=f"xt{i}")
                nc.scalar.dma_start_transpose(
                    xt[:], x16[n0:n0 + NB, 128 * i:128 * (i + 1)])
                xts.append(xt)

            # LoRA-A projections t_g, t_u : [16, NB]
            tg = small.tile([16, NB], F16, tag=f"tg{nb}")
            tu = small.tile([16, NB], F16, tag=f"tu{nb}")
            for k, (la_s, tout) in enumerate(((gla_s, tg), (ula_s, tu))):
                for h in range(NH):
                    pl = ptile(2 * k + h, 16)
                    for i in range(IT):
                        nc.tensor.matmul(pl, la_s[:, i, :],
                                         xts[i][:, bass.ts(h, NF)],
                                         start=(i == 0), stop=(i == IT - 1))
                    nc.scalar.copy(tout[:, bass.ts(h, NF)], pl)

            # gate/up GEMMs + gating, per j group of 2: one pass over i with
            # gate and up interleaved; gating = relu(pg)*pu in one DVE op.
            x3s = []
            for (g0, jn) in JG:
                W = 128 * jn
                pg = [[ptile(h * jn + jj) for jj in range(jn)] for h in range(NH)]
                pu = [[ptile(4 + h * jn + jj) for jj in range(jn)] for h in range(NH)]
                for i in range(IT):
                    wgt = wstr.tile([128, 256], F16, tag="wgt", name="wgt")[:, :W]
                    nc.sync.dma_start(
                        wgt, wgd[128 * i:128 * (i + 1), 128 * g0:128 * g0 + W])
                    wut = wstr.tile([128, 256], F16, tag="wut", name="wut")[:, :W]
                    nc.sync.dma_start(
                        wut, wud[128 * i:128 * (i + 1), 128 * g0:128 * g0 + W])
                    for jj in range(jn):
                        for h in range(NH):
                            nc.tensor.matmul(pg[h][jj], wgt[:, bass.ts(jj, 128)],
                                             xts[i][:, bass.ts(h, NF)],
                                             start=(i == 0), stop=False)
                        for h in range(NH):
                            nc.tensor.matmul(pu[h][jj], wut[:, bass.ts(jj, 128)],
                                             xts[i][:, bass.ts(h, NF)],
                                             start=(i == 0), stop=False)
                for jj in range(jn):
                    j = g0 + jj
                    x3 = x3blk.tile([128, NB], F16, tag=f"x3_{j}", name=f"x3_{j}")
                    for h in range(NH):
                        nc.tensor.matmul(pg[h][jj], glb_s[:, 128 * j:128 * (j + 1)],
                                         tg[:, bass.ts(h, NF)], start=False, stop=True)
                        nc.tensor.matmul(pu[h][jj], ulb_s[:, 128 * j:128 * (j + 1)],
                                         tu[:, bass.ts(h, NF)], start=False, stop=True)
                        # x3 = relu(pg) * pu  (relu on ACT, mul on DVE -
                        # a DVE op may read at most one PSUM operand)
                        r = evac.tile([128, NF], F16, tag="relu", name="relu")
                        nc.scalar.activation(r[:], pg[h][jj], AF.Relu,
                                             bias=0.0, scale=1.0)
                        nc.vector.tensor_tensor(x3[:, bass.ts(h, NF)], r[:],
                                                pu[h][jj], OP.mult)
                    x3s.append(x3)

            # LoRA-A down projection t_d : [16, NB]
            td = small.tile([16, NB], F16, tag=f"td{nb}")
            for h in range(NH):
                pl = ptile(h, 16)
                for j in range(JT):
                    nc.tensor.matmul(pl, dla_s[:, j, :], x3s[j][:, bass.ts(h, NF)],
                                     start=(j == 0), stop=(j == JT - 1))
                nc.scalar.copy(td[:, bass.ts(h, NF)], pl)

            # down GEMM: y3[n, m] partial (rotate psum slots for ILP)
            for m in range(MT):
                wds = []
                for j in range(JT):
                    wd = wdstr.tile([128, NF], F16, tag="wd")
                    nc.sync.dma_start(
                        wd[:], wdd[128 * j:128 * (j + 1), bass.ts(m, NF)])
                    wds.append(wd)
                for ns in range(NS):
                    pd = ptile((m * NS + ns) % 8)
                    for j in range(JT):
                        nc.tensor.matmul(pd, x3s[j][:, bass.ts(ns, 128)], wds[j][:],
                                         start=(j == 0), stop=False)
                    nc.tensor.matmul(pd, td[:, bass.ts(ns, 128)],
                                     dlb_s[:, bass.ts(m, NF)], start=False, stop=True)
                    res = evac.tile([128, NF], F16, tag="res")
                    nc.scalar.copy(res[:], pd)
                    nc.scalar.dma_start(
                        y3p[n0 + 128 * ns:n0 + 128 * (ns + 1), bass.ts(m, NF)],
                        res[:])

    nc.finalize()
    return nc


# ---------------------------------------------------------------------------
# host side
# ---------------------------------------------------------------------------

_CACHED_NC = {}


def _get_nc(N_TOK, DM, JS, NB):
    key = (N_TOK, DM, JS, NB)
    if key not in _CACHED_NC:
        _CACHED_NC[key] = build_nc(N_TOK, DM, JS, NB)
    return _CACHED_NC[key]


def make_in_maps(x1, g_codes, g_absmax, g_lora_a, g_lora_b,
                 u_codes, u_absmax, u_lora_a, u_lora_b,
                 d_codes, d_absmax, d_lora_a, d_lora_b, ncores):
    """Shard + marshal inputs for the SPMD kernel."""
    DM = x1.shape[-1]
    N_TOK = int(np.prod(x1.shape[:-1]))
    DFF = g_codes.shape[0]
    JS = ((DFF + ncores * BLOCK - 1) // (ncores * BLOCK)) * BLOCK
    JBS = JS // BLOCK

    x16 = np.ascontiguousarray(x1.reshape(N_TOK, DM)).astype(np.float16)

    def pad_rows(a, n):
        if a.shape[0] >= n:
            return a[:n]
        return np.concatenate(
            [a, np.zeros((n - a.shape[0],) + a.shape[1:], a.dtype)], axis=0)

    def pad_cols(a, n):
        if a.shape[1] >= n:
            return a[:, :n]
        return np.concatenate(
            [a, np.zeros((a.shape[0], n - a.shape[1]), a.dtype)], axis=1)

    in_maps = []
    for c in range(ncores):
        js, je = c * JS, (c + 1) * JS
        jb, jbe = c * JBS, (c + 1) * JBS
        gc = pad_rows(g_codes[js:je], JS).astype(np.int16)
        ga = pad_rows(g_absmax[js:je], JS).T.astype(np.float16)  # [DM//B, JS]
        ga = np.ascontiguousarray(ga)
        ucs = pad_rows(u_codes[js:je], JS).astype(np.int16)
        ua = np.ascontiguousarray(pad_rows(u_absmax[js:je], JS).T.astype(np.float16))
        dcs = pad_cols(d_codes[:, js:je], JS).astype(np.int16)
        da = np.ascontiguousarray(
            pad_cols(d_absmax[:, jb:jbe], JBS).T.astype(np.float16))  # [JBS, DM]
        in_maps.append(dict(
            x16=x16,
            gc=np.ascontiguousarray(gc), gam=ga,
            uc=np.ascontiguousarray(ucs), uam=ua,
            dc=np.ascontiguousarray(dcs), dam=da,
            gla=g_lora_a.astype(np.float16),
            glb=np.ascontiguousarray(pad_cols(g_lora_b[:, js:je], JS).astype(np.float16)),
            ula=u_lora_a.astype(np.float16),
            ulb=np.ascontiguousarray(pad_cols(u_lora_b[:, js:je], JS).astype(np.float16)),
            dla=np.ascontiguousarray(pad_rows(d_lora_a[js:je], JS).astype(np.float16)),
            dlb=d_lora_b.astype(np.float16),
        ))
    return in_maps, N_TOK, DM, JS


def kernel(x1, g_codes, g_absmax, g_lora_a, g_lora_b,
           u_codes, u_absmax, u_lora_a, u_lora_b,
           d_codes, d_absmax, d_lora_a, d_lora_b,
           _trace=False):
    from concourse.bass_utils import run_bass_kernel_spmd

    in_maps, N_TOK, DM, JS = make_in_maps(
        x1, g_codes, g_absmax, g_lora_a, g_lora_b,
        u_codes, u_absmax, u_lora_a, u_lora_b,
        d_codes, d_absmax, d_lora_a, d_lora_b, NCORES)
    NB = 1024 if N_TOK % 1024 == 0 else N_TOK
    nc = _get_nc(N_TOK, DM, JS, NB)

    res = run_bass_kernel_spmd(nc, in_maps, core_ids=list(range(NCORES)),
                               trace=_trace)
    acc = np.zeros((N_TOK, DM), np.float32)
    for r in res.results:
        acc += r["y3p"].astype(np.float32)
    out = acc.reshape(x1.shape).astype(np.float32)
    if _trace:
        kernel._last_exec_time_ns = res.exec_time_ns
        kernel._last_results = res
    return out


# revision 16
# speedup vs baseline: 1.3735x; 1.3735x over previous
"""Trainium2 Bass kernel for nn_MixedSparseGatedMLP (QLoRA-style NF4 gated MLP).

  y1 = x @ dequant(g).T + (x @ g_lora_a) @ g_lora_b
  y2 = x @ dequant(u).T + (x @ u_lora_a) @ u_lora_b
  y3 = (relu(y1)*y2) @ dequant(d).T + ((relu(y1)*y2) @ d_lora_a) @ d_lora_b

Strategy (Megatron tensor-parallel over d_ff across 8 cores):
 - d_ff padded 11008 -> 11264 = 8 * 1408; each core owns a 1408-wide shard of
   gate/up output channels (and the matching rows of the down projection).
   Padded channels carry absmax = 0 so they contribute exactly zero.
 - NF4 dequant on device: the 16-entry codebook is approximated by
   s1*ln(c+a) - s2*ln(b-c) + d0 + A1*sin(B1 c+P1) + A2*tanh(B2 c+P2)
   (max abs err 1.4e-3, rel rms 0.12%) evaluated on the ScalarEngine with
   4 activation passes + 5 VectorEngine combine passes, entirely overlapped
   with TensorEngine GEMMs. absmax scaling via partition-broadcast DMA.
 - Codes are uploaded pre-transposed-ready as int16; transposing DMA loads
   produce codes^T tiles so dequant directly yields W' in [in, out] layout
   (the layout the PE needs) - no on-device weight transpose.
 - LoRA adapters fold in as extra K=16 accumulation matmuls (concat trick).
 - fp16 on-device compute (PSUM accumulates fp32); each core returns its
   partial y3 in fp16; host reduces the 8 partials in fp32.
"""

import sys

sys.path.insert(0, "/opt/trn_rl_repo")

from contextlib import ExitStack

import numpy as np

import concourse.bass as bass
import concourse.mybir as mybir
import concourse.tile as tile
from concourse import bacc

F16 = mybir.dt.float16
F32 = mybir.dt.float32
I16 = mybir.dt.int16
AF = mybir.ActivationFunctionType
OP = mybir.AluOpType

# fitted NF4 approximation params (logit + sin + tanh), see module docstring
S1, LA, S2, LB, D0, A1, B1, P1, A2, B2, P2 = [
    3.20329446e-01, 6.94421837e-01, 3.37825232e-01, 1.58366395e+01,
    6.40217907e-02, 3.23956362e-02, 3.36126831e-01, -2.12588407e+00,
    -1.35181599e-02, 7.56234953e-01, -6.20180727e+00]

BLOCK = 64
NCORES = 8


def build_nc(N_TOK, DM, JS, NB):
    """One-core SPMD program. JS = per-core d_ff shard width, NB = token block.

    Device inputs (per core):
      x16   [N_TOK, DM] f16      - tokens (replicated)
      gc,uc [JS, DM]   i16       - gate/up NF4 codes, shard rows
      gam,uam [DM//BLOCK, JS] f16 - absmax transposed
      dc    [DM, JS]   i16       - down codes, shard cols
      dam   [JS//BLOCK, DM] f16  - down absmax transposed
      gla,ula [DM, 16] f16; glb,ulb [16, JS] f16
      dla   [JS, 16]   f16; dlb [16, DM] f16
    Output: y3p [N_TOK, DM] f16 - this core's partial down projection
    """
    IT = DM // 128          # i tiles (contraction dim of gate/up)
    JT = JS // 128          # j tiles (shard channels)
    NBLK = N_TOK // NB      # token blocks
    NF = min(512, NB)       # matmul moving free dim
    NH = NB // NF           # halves per block
    NS = NB // 128          # 128-token subtiles per block
    MT = DM // NF           # output column tiles (down)
    DQF = min(1024, DM)     # free-dim chunk for down dequant
    DQC = DM // DQF
    BPT = 128 // BLOCK      # absmax rows per 128-row tile

    nc = bacc.Bacc("TRN2", target_bir_lowering=False, debug=False,
                   num_devices=NCORES)

    x16 = nc.dram_tensor("x16", [N_TOK, DM], F16, kind="ExternalInput")
    gc = nc.dram_tensor("gc", [JS, DM], I16, kind="ExternalInput")
    gam = nc.dram_tensor("gam", [DM // BLOCK, JS], F16, kind="ExternalInput")
    uc = nc.dram_tensor("uc", [JS, DM], I16, kind="ExternalInput")
    uam = nc.dram_tensor("uam", [DM // BLOCK, JS], F16, kind="ExternalInput")
    dc = nc.dram_tensor("dc", [DM, JS], I16, kind="ExternalInput")
    dam = nc.dram_tensor("dam", [JS // BLOCK, DM], F16, kind="ExternalInput")
    gla = nc.dram_tensor("gla", [DM, 16], F16, kind="ExternalInput")
    glb = nc.dram_tensor("glb", [16, JS], F16, kind="ExternalInput")
    ula = nc.dram_tensor("ula", [DM, 16], F16, kind="ExternalInput")
    ulb = nc.dram_tensor("ulb", [16, JS], F16, kind="ExternalInput")
    dla = nc.dram_tensor("dla", [JS, 16], F16, kind="ExternalInput")
    dlb = nc.dram_tensor("dlb", [16, DM], F16, kind="ExternalInput")
    y3p = nc.dram_tensor("y3p", [N_TOK, DM], F16, kind="ExternalOutput")

    with tile.TileContext(nc) as tc, ExitStack() as ctx:
        dram = ctx.enter_context(tc.tile_pool(name="dram", bufs=1, space="DRAM"))
        small = ctx.enter_context(tc.tile_pool(name="small", bufs=1))

        wgd = dram.tile([DM, JS], F16)   # W' gate  [in, out]
        wud = dram.tile([DM, JS], F16)   # W' up    [in, out]
        wdd = dram.tile([JS, DM], F16)   # W' down  [in(j), out(m)]

        # activation bias constants
        biases = small.tile([128, 4], F32)
        for k, v in enumerate([LA, LB, P1, P2]):
            nc.vector.memset(biases[:, k:k + 1], float(v))
        b_la, b_lb, b_p1, b_p2 = (biases[:, k:k + 1] for k in range(4))

        # resident LoRA factors
        gla_s = small.tile([128, IT, 16], F16)
        nc.sync.dma_start(gla_s[:], gla.rearrange("(t p) r -> p t r", p=128))
        ula_s = small.tile([128, IT, 16], F16)
        nc.sync.dma_start(ula_s[:], ula.rearrange("(t p) r -> p t r", p=128))
        dla_s = small.tile([128, JT, 16], F16)
        nc.sync.dma_start(dla_s[:], dla.rearrange("(t p) r -> p t r", p=128))
        glb_s = small.tile([16, JS], F16)
        nc.sync.dma_start(glb_s[:], glb[:])
        ulb_s = small.tile([16, JS], F16)
        nc.sync.dma_start(ulb_s[:], ulb[:])
        dlb_s = small.tile([16, DM], F16)
        nc.sync.dma_start(dlb_s[:], dlb[:])

        # ---- dequant: codes^T tile [128, F] -> W' tile, written to DRAM ----
        def dequant_tile(pool, codes_dram, am_dram, w_dram, t, F, f0):
            """rows [128t, 128t+128) of W' (= input-dim), free cols [f0, f0+F)"""
            ct = pool.tile([128, F], I16, tag="ct")
            nc.sync.dma_start_transpose(
                ct[:], codes_dram[f0:f0 + F, 128 * t:128 * (t + 1)])
            am_b = pool.tile([128, F], F16, tag="am_b")
            for b in range(BPT):
                nc.gpsimd.dma_start(
                    am_b[BLOCK * b:BLOCK * (b + 1), :],
                    am_dram[BPT * t + b:BPT * t + b + 1, f0:f0 + F]
                    .broadcast_to([BLOCK, F]))
            u1 = pool.tile([128, F], F16, tag="u1")
            u2 = pool.tile([128, F], F16, tag="u2")
            u3 = pool.tile([128, F], F16, tag="u3")
            u4 = pool.tile([128, F], F16, tag="u4")
            nc.scalar.activation(u1[:], ct[:], AF.Ln, bias=b_la, scale=1.0)
            nc.scalar.activation(u2[:], ct[:], AF.Ln, bias=b_lb, scale=-1.0)
            nc.scalar.activation(u3[:], ct[:], AF.Sin, bias=b_p1, scale=float(B1))
            nc.scalar.activation(u4[:], ct[:], AF.Tanh, bias=b_p2, scale=float(B2))
            t1 = pool.tile([128, F], F16, tag="t1")
            nc.vector.scalar_tensor_tensor(
                t1[:], u1[:], float(S1 / S2), u2[:], OP.mult, OP.subtract)
            t2 = pool.tile([128, F], F16, tag="t2")
            nc.vector.scalar_tensor_tensor(
                t2[:], u3[:], float(A1 / A2), u4[:], OP.mult, OP.add)
            t3 = pool.tile([128, F], F16, tag="t3")
            nc.vector.scalar_tensor_tensor(
                t3[:], t1[:], float(S2 / A2), t2[:], OP.mult, OP.add)
            t4 = pool.tile([128, F], F16, tag="t4")
            nc.vector.tensor_scalar(
                t4[:], t3[:], float(A2), float(D0), OP.mult, OP.add)
            wt = pool.tile([128, F], F16, tag="wt")
            nc.vector.tensor_tensor(wt[:], t4[:], am_b[:], OP.mult)
            nc.gpsimd.dma_start(w_dram[128 * t:128 * (t + 1), f0:f0 + F], wt[:])

        with tc.tile_pool(name="deq", bufs=4) as deq:
            for t in range(IT):
                dequant_tile(deq, gc, gam, wgd, t, JS, 0)
            for t in range(IT):
                dequant_tile(deq, uc, uam, wud, t, JS, 0)
            for t in range(JT):
                for ch in range(DQC):
                    dequant_tile(deq, dc, dam, wdd, t, DQF, ch * DQF)

        # ---- GEMM chain ----
        xblk = ctx.enter_context(tc.tile_pool(name="xblk", bufs=1))
        x3blk = ctx.enter_context(tc.tile_pool(name="x3blk", bufs=1))
        rblk = ctx.enter_context(tc.tile_pool(name="rblk", bufs=1))
        wstr = ctx.enter_context(tc.tile_pool(name="wstr", bufs=8))
        wdstr = ctx.enter_context(tc.tile_pool(name="wdstr", bufs=JT + 3))
        evac = ctx.enter_context(tc.tile_pool(name="evac", bufs=8))
        psum = ctx.enter_context(tc.tile_pool(name="psum", bufs=1, space="PSUM"))

        # all PSUM usage shares one 8-slot tag family (one bank each)
        def ptile(idx, p=128):
            return psum.tile([128, NF], F32, tag=f"pp{idx}", name=f"pp{idx}")[:p]

        # j-tile groups of up to 4 (gate pass: 4 j x 2 halves = 8 banks, then
        # relu-evacuated before the up pass reuses the same banks)
        JG = []
        j0 = 0
        while j0 < JT:
            jn = min(4, JT - j0)
            JG.append((j0, jn))
            j0 += jn

        for nb in range(NBLK):
            n0 = nb * NB
            # resident x^T tiles for this block
            xts = []
            for i in range(IT):
                xt = xblk.tile([128, NB], F16, tag=f"xt{i}", name=f"xt{i}")
                nc.scalar.dma_start_transpose(
                    xt[:], x16[n0:n0 + NB, 128 * i:128 * (i + 1)])
                xts.append(xt)

            # LoRA-A projections t_g, t_u : [16, NB]
            tg = small.tile([16, NB], F16, tag=f"tg{nb}")
            tu = small.tile([16, NB], F16, tag=f"tu{nb}")
            for k, (la_s, tout) in enumerate(((gla_s, tg), (ula_s, tu))):
                for h in range(NH):
                    pl = ptile(2 * k + h, 16)
                    for i in range(IT):
                        nc.tensor.matmul(pl, la_s[:, i, :],
                                         xts[i][:, bass.ts(h, NF)],
                                         start=(i == 0), stop=(i == IT - 1))
                    nc.scalar.copy(tout[:, bass.ts(h, NF)], pl)

            # gate/up GEMMs + gating, per j group: gate pass -> relu evac ->
            # up pass -> gating mult. W loads are [128, jn*128] fat lines.
            x3s = []
            for (g0, jn) in JG:
                W = 128 * jn
                pg = [[ptile(h * 4 + jj) for jj in range(jn)] for h in range(NH)]
                for i in range(IT):
                    wgt = wstr.tile([128, 512], F16, tag="wgt", name="wgt")[:, :W]
                    nc.sync.dma_start(
                        wgt, wgd[128 * i:128 * (i + 1), 128 * g0:128 * g0 + W])
                    for jj in range(jn):
                        for h in range(NH):
                            nc.tensor.matmul(pg[h][jj], wgt[:, bass.ts(jj, 128)],
                                             xts[i][:, bass.ts(h, NF)],
                                             start=(i == 0), stop=False)
                rs = []
                for jj in range(jn):
                    j = g0 + jj
                    r = rblk.tile([128, NB], F16, tag=f"r{jj}", name=f"r{jj}")
                    for h in range(NH):
                        nc.tensor.matmul(pg[h][jj], glb_s[:, 128 * j:128 * (j + 1)],
                                         tg[:, bass.ts(h, NF)], start=False, stop=True)
                        nc.scalar.activation(r[:, bass.ts(h, NF)], pg[h][jj],
                                             AF.Relu, bias=0.0, scale=1.0)
                    rs.append(r)
                # up pass (reuses the same psum slots; Tile orders via WAR)
                pu = [[ptile(h * 4 + jj) for jj in range(jn)] for h in range(NH)]
                for i in range(IT):
                    wut = wstr.tile([128, 512], F16, tag="wut", name="wut")[:, :W]
                    nc.sync.dma_start(
                        wut, wud[128 * i:128 * (i + 1), 128 * g0:128 * g0 + W])
                    for jj in range(jn):
                        for h in range(NH):
                            nc.tensor.matmul(pu[h][jj], wut[:, bass.ts(jj, 128)],
                                             xts[i][:, bass.ts(h, NF)],
                                             start=(i == 0), stop=False)
                for jj in range(jn):
                    j = g0 + jj
                    x3 = x3blk.tile([128, NB], F16, tag=f"x3_{j}", name=f"x3_{j}")
                    for h in range(NH):
                        nc.tensor.matmul(pu[h][jj], ulb_s[:, 128 * j:128 * (j + 1)],
                                         tu[:, bass.ts(h, NF)], start=False, stop=True)
                        nc.vector.tensor_tensor(x3[:, bass.ts(h, NF)],
                                                rs[jj][:, bass.ts(h, NF)],
                                                pu[h][jj], OP.mult)
                    x3s.append(x3)

            # LoRA-A down projection t_d : [16, NB]
            td = small.tile([16, NB], F16, tag=f"td{nb}")
            for h in range(NH):
                pl = ptile(h, 16)
                for j in range(JT):
                    nc.tensor.matmul(pl, dla_s[:, j, :], x3s[j][:, bass.ts(h, NF)],
                                     start=(j == 0), stop=(j == JT - 1))
                nc.scalar.copy(td[:, bass.ts(h, NF)], pl)

            # down GEMM: y3[n, m] partial (rotate psum slots for ILP; evac on
            # DVE, store via the gpsimd queue which is idle by now)
            for m in range(MT):
                wds = []
                for j in range(JT):
                    wd = wdstr.tile([128, NF], F16, tag="wd")
                    nc.sync.dma_start(
                        wd[:], wdd[128 * j:128 * (j + 1), bass.ts(m, NF)])
                    wds.append(wd)
                for ns in range(NS):
                    pd = ptile((m * NS + ns) % 8)
                    for j in range(JT):
                        nc.tensor.matmul(pd, x3s[j][:, bass.ts(ns, 128)], wds[j][:],
                                         start=(j == 0), stop=False)
                    nc.tensor.matmul(pd, td[:, bass.ts(ns, 128)],
                                     dlb_s[:, bass.ts(m, NF)], start=False, stop=True)
                    res = evac.tile([128, NF], F16, tag="res")
                    nc.vector.tensor_copy(res[:], pd)
                    nc.gpsimd.dma_start(
                        y3p[n0 + 128 * ns:n0 + 128 * (ns + 1), bass.ts(m, NF)],
                        res[:])

    nc.finalize()
    return nc


# ---------------------------------------------------------------------------
# host side
# ---------------------------------------------------------------------------

_CACHED_NC = {}


def _get_nc(N_TOK, DM, JS, NB):
    key = (N_TOK, DM, JS, NB)
    if key not in _CACHED_NC:
        _CACHED_NC[key] = build_nc(N_TOK, DM, JS, NB)
    return _CACHED_NC[key]


def make_in_maps(x1, g_codes, g_absmax, g_lora_a, g_lora_b,
                 u_codes, u_absmax, u_lora_a, u_lora_b,
                 d_codes, d_absmax, d_lora_a, d_lora_b, ncores):
    """Shard + marshal inputs for the SPMD kernel."""
    DM = x1.shape[-1]
    N_TOK = int(np.prod(x1.shape[:-1]))
    DFF = g_codes.shape[0]
    JS = ((DFF + ncores * BLOCK - 1) // (ncores * BLOCK)) * BLOCK
    JBS = JS // BLOCK

    x16 = np.ascontiguousarray(x1.reshape(N_TOK, DM)).astype(np.float16)

    def pad_rows(a, n):
        if a.shape[0] >= n:
            return a[:n]
        return np.concatenate(
            [a, np.zeros((n - a.shape[0],) + a.shape[1:], a.dtype)], axis=0)

    def pad_cols(a, n):
        if a.shape[1] >= n:
            return a[:, :n]
        return np.concatenate(
            [a, np.zeros((a.shape[0], n - a.shape[1]), a.dtype)], axis=1)

    in_maps = []
    for c in range(ncores):
        js, je = c * JS, (c + 1) * JS
        jb, jbe = c * JBS, (c + 1) * JBS
        gc = pad_rows(g_codes[js:je], JS).astype(np.int16)
        ga = pad_rows(g_absmax[js:je], JS).T.astype(np.float16)  # [DM//B, JS]
        ga = np.ascontiguousarray(ga)
        ucs = pad_rows(u_codes[js:je], JS).astype(np.int16)
        ua = np.ascontiguousarray(pad_rows(u_absmax[js:je], JS).T.astype(np.float16))
        dcs = pad_cols(d_codes[:, js:je], JS).astype(np.int16)
        da = np.ascontiguousarray(
            pad_cols(d_absmax[:, jb:jbe], JBS).T.astype(np.float16))  # [JBS, DM]
        in_maps.append(dict(
            x16=x16,
            gc=np.ascontiguousarray(gc), gam=ga,
            uc=np.ascontiguousarray(ucs), uam=ua,
            dc=np.ascontiguousarray(dcs), dam=da,
            gla=g_lora_a.astype(np.float16),
            glb=np.ascontiguousarray(pad_cols(g_lora_b[:, js:je], JS).astype(np.float16)),
            ula=u_lora_a.astype(np.float16),
            ulb=np.ascontiguousarray(pad_cols(u_lora_b[:, js:je], JS).astype(np.float16)),
            dla=np.ascontiguousarray(pad_rows(d_lora_a[js:je], JS).astype(np.float16)),
            dlb=d_lora_b.astype(np.float16),
        ))
    return in_maps, N_TOK, DM, JS


def kernel(x1, g_codes, g_absmax, g_lora_a, g_lora_b,
           u_codes, u_absmax, u_lora_a, u_lora_b,
           d_codes, d_absmax, d_lora_a, d_lora_b,
           _trace=False):
    from concourse.bass_utils import run_bass_kernel_spmd

    in_maps, N_TOK, DM, JS = make_in_maps(
        x1, g_codes, g_absmax, g_lora_a, g_lora_b,
        u_codes, u_absmax, u_lora_a, u_lora_b,
        d_codes, d_absmax, d_lora_a, d_lora_b, NCORES)
    NB = 1024 if N_TOK % 1024 == 0 else N_TOK
    nc = _get_nc(N_TOK, DM, JS, NB)

    res = run_bass_kernel_spmd(nc, in_maps, core_ids=list(range(NCORES)),
                               trace=_trace)
    acc = np.zeros((N_TOK, DM), np.float32)
    for r in res.results:
        acc += r["y3p"].astype(np.float32)
    out = acc.reshape(x1.shape).astype(np.float32)
    if _trace:
        kernel._last_exec_time_ns = res.exec_time_ns
        kernel._last_results = res
    return out
